# revision 2
# baseline (speedup 1.0000x reference)
"""TopK sparse autoencoder forward pass on 8 TRN2 NeuronCores.

Data-parallel over the batch: each core owns 512 rows and runs an identical
program (SPMD).  Per core:

  A. encode:  acts = relu((x - b_dec) @ W_enc.T + b_enc)   [fp32 matmul]
     - W_encT streamed from HBM once; acts spilled to DRAM in fp32
     - per-256-chunk top-8 candidates extracted from drain bounces (DVE max8)
  B. topk:    exact top-k threshold tau from the candidate array via
     iterated max8 + match_replace; exactness flag per row
  C. mask:    enc = (acts >= tau) * acts, cast bf16, DMA-transposed to [F, B]
  D. decode:  x_hat = enc @ W_dec.T + b_dec                [bf16 matmul]

The per-chunk top-8 candidate set provably contains the true top-k unless
some 256-wide chunk holds >8 of the top-k values; that condition is detected
on-device (flag = chunk-8th-largest > tau) and the handful of flagged rows
(expected: zero) are recomputed exactly on the host.
"""

import numpy as np
import ml_dtypes

ACT_DIM = 768
DICT = 24576
BATCH = 4096
NCORES = 8
ROWS = BATCH // NCORES          # 512 rows per core
NT = ROWS // 128                # 4 row-tiles per core
CH = 256                        # stage-1 chunk width
NCH = DICT // CH                # 96 chunks
CANDW = NCH * 8                 # 768 candidates per row
NEG = -1.0e30
BF16 = ml_dtypes.bfloat16

_cache = {}


def _build(k: int, with_benc: bool):
    import concourse.bass as bass
    import concourse.mybir as mybir
    from concourse import bacc
    from concourse import tile

    f32 = mybir.dt.float32
    bf16 = mybir.dt.bfloat16
    ROUNDS = (k + 7) // 8

    nc = bacc.Bacc("TRN2", target_bir_lowering=False, debug=False,
                   num_devices=NCORES)

    xT_d = nc.dram_tensor("xT", [ACT_DIM, ROWS], f32, kind="ExternalInput")
    wencT_d = nc.dram_tensor("wencT", [ACT_DIM, DICT], f32, kind="ExternalInput")
    wdecT_d = nc.dram_tensor("wdecT", [DICT, ACT_DIM], bf16, kind="ExternalInput")
    bdec_d = nc.dram_tensor("bdec", [1, ACT_DIM], f32, kind="ExternalInput")
    if with_benc:
        benc_d = nc.dram_tensor("benc", [1, DICT], f32, kind="ExternalInput")
    xhat_d = nc.dram_tensor("xhat", [ROWS, ACT_DIM], f32, kind="ExternalOutput")
    flags_d = nc.dram_tensor("flags", [128, NT], f32, kind="ExternalOutput")
    acts_spill = nc.dram_tensor("acts_spill", [NT, 128, DICT], f32)

    NSC = DICT // 512           # 48 encode column-chunks
    NBLK = DICT // 2048         # 12 mask/transpose blocks
    NFG = DICT // 1024          # 24 decoder stream groups (8 f-chunks each)
    NF = DICT // 128            # 192 decoder f-chunks

    with tile.TileContext(nc) as tc:
        with tc.tile_pool(name="const", bufs=1) as constp, \
             tc.tile_pool(name="cand", bufs=NT) as candp, \
             tc.tile_pool(name="small", bufs=4 * NT + 4) as smallp:

            bdec_row = constp.tile([1, ACT_DIM], f32)
            nc.sync.dma_start(bdec_row[:], bdec_d.ap())
            bdec_bc = constp.tile([128, ACT_DIM], f32)
            nc.gpsimd.partition_broadcast(bdec_bc[:], bdec_row[:])
            if with_benc:
                benc_row = constp.tile([1, DICT], f32)
                nc.sync.dma_start(benc_row[:], benc_d.ap())

            flags_sb = constp.tile([128, NT], f32)
            cands = [candp.tile([128, CANDW], f32, tag="cand", name=f"cand{t}")
                     for t in range(NT)]
            taus = [smallp.tile([128, 1], f32, tag="tau", name=f"tau{t}")
                    for t in range(NT)]

            # ---------------- Phase A: encode + spill + stage-1 ----------
            with tc.tile_pool(name="xt", bufs=1) as xtp, \
                 tc.tile_pool(name="wenc", bufs=4) as wencp, \
                 tc.tile_pool(name="bounce", bufs=6) as bouncep, \
                 tc.tile_pool(name="encpsum", bufs=6, space="PSUM") as encpsp, \
                 tc.tile_pool(name="bencbc", bufs=2) as bencbcp:

                xT_sb = xtp.tile([128, ACT_DIM // 128, ROWS], f32)
                nc.sync.dma_start(
                    xT_sb[:], xT_d.ap().rearrange("(a p) r -> p a r", p=128))

                for sc in range(NSC):
                    wt = wencp.tile([128, ACT_DIM // 128, 512], f32, tag="wenc")
                    nc.sync.dma_start(
                        wt[:],
                        wencT_d.ap()[:, sc * 512:(sc + 1) * 512]
                        .rearrange("(a p) c -> p a c", p=128))
                    if with_benc:
                        bb = bencbcp.tile([128, 512], f32, tag="bb")
                        nc.gpsimd.partition_broadcast(
                            bb[:], benc_row[0:1, sc * 512:(sc + 1) * 512])
                    for t in range(NT):
                        ps = encpsp.tile([128, 512], f32, tag="eps")
                        for a in range(ACT_DIM // 128):
                            nc.tensor.matmul(
                                ps[:],
                                xT_sb[:, a, t * 128:(t + 1) * 128],
                                wt[:, a, :],
                                start=(a == 0), stop=(a == ACT_DIM // 128 - 1))
                        bo = bouncep.tile([128, 512], f32, tag="bo")
                        if with_benc:
                            nc.vector.tensor_tensor(bo[:], ps[:], bb[:],
                                                    op=mybir.AluOpType.add)
                            nc.scalar.activation(
                                bo[:], bo[:], mybir.ActivationFunctionType.Relu)
                        else:
                            nc.scalar.activation(
                                bo[:], ps[:], mybir.ActivationFunctionType.Relu)
                        nc.sync.dma_start(
                            acts_spill.ap()[t, :, sc * 512:(sc + 1) * 512], bo[:])
                        for cc in range(512 // CH):
                            c = sc * (512 // CH) + cc
                            nc.vector.max(
                                cands[t][:, c * 8:(c + 1) * 8],
                                bo[:, cc * CH:(cc + 1) * CH])

            # ---------------- Phase B: exact top-k threshold -------------
            for t in range(NT):
                c8 = smallp.tile([128, 1], f32, tag="c8")
                cand3 = cands[t][:].rearrange("p (c e) -> p c e", e=8)
                nc.vector.tensor_reduce(c8[:], cand3[:, :, 7:8],
                                        axis=mybir.AxisListType.XY,
                                        op=mybir.AluOpType.max)
                topv = smallp.tile([128, 8 * ROUNDS], f32, tag="topv")
                for r in range(ROUNDS):
                    nc.vector.max(topv[:, r * 8:(r + 1) * 8], cands[t][:])
                    if r < ROUNDS - 1:
                        nc.vector.match_replace(
                            cands[t][:], topv[:, r * 8:(r + 1) * 8],
                            cands[t][:], NEG)
                nc.vector.tensor_copy(taus[t][:], topv[:, k - 1:k])
                nc.vector.tensor_tensor(flags_sb[:, t:t + 1], c8[:], taus[t][:],
                                        op=mybir.AluOpType.is_gt)
            nc.sync.dma_start(flags_d.ap(), flags_sb[:])

            # ---------------- Phases C+D: mask, transpose, decode --------
            with tc.tile_pool(name="actsc", bufs=3) as actscp, \
                 tc.tile_pool(name="encb", bufs=3) as encbp, \
                 tc.tile_pool(name="enct", bufs=4) as enctp, \
                 tc.tile_pool(name="wdec", bufs=3) as wdecp, \
                 tc.tile_pool(name="decpsum", bufs=4, space="PSUM") as decpsp, \
                 tc.tile_pool(name="outsb", bufs=2) as outp:

                for pair in range(NT // 2):
                    tiles = (2 * pair, 2 * pair + 1)
                    # C: mask + transpose into two half-F stationary tensors
                    encts = {}
                    for t in tiles:
                        encts[t] = [enctp.tile([128, NF // 2, 128], bf16,
                                               tag="enct", name=f"enct{t}_{h}")
                                    for h in range(2)]
                        for blk in range(NBLK):
                            ac = actscp.tile([128, 2048], f32, tag="ac")
                            nc.sync.dma_start(
                                ac[:],
                                acts_spill.ap()[t, :, blk * 2048:(blk + 1) * 2048])
                            eb = encbp.tile([128, 2048], bf16, tag="eb")
                            nc.vector.scalar_tensor_tensor(
                                eb[:], ac[:], taus[t][:, 0:1], ac[:],
                                op0=mybir.AluOpType.is_ge,
                                op1=mybir.AluOpType.mult)
                            half, fo = divmod(blk * 16, NF // 2)
                            nc.sync.dma_start_transpose(
                                encts[t][half][:, fo:fo + 16, :], eb[:])
                    # D: decode pair with W_dec streamed once
                    pss = {t: decpsp.tile([128, ACT_DIM], f32, tag="dps",
                                           name=f"dps{t}")
                           for t in tiles}
                    for fg in range(NFG):
                        wd = wdecp.tile([128, 8, ACT_DIM], bf16, tag="wd")
                        nc.sync.dma_start(
                            wd[:],
                            wdecT_d.ap()[fg * 1024:(fg + 1) * 1024, :]
                            .rearrange("(c p) a -> p c a", p=128))
                        for t in tiles:
                            for j in range(8):
                                f = fg * 8 + j
                                half, fo = divmod(f, NF // 2)
                                lhsT = encts[t][half][:, fo, :]
                                st = (f == 0)
                                sp = (f == NF - 1)
                                nc.tensor.matmul(
                                    pss[t][:, 0:512], lhsT, wd[:, j, 0:512],
                                    start=st, stop=sp)
                                nc.tensor.matmul(
                                    pss[t][:, 512:ACT_DIM], lhsT,
                                    wd[:, j, 512:ACT_DIM],
                                    start=st, stop=sp)
                    for t in tiles:
                        ot = outp.tile([128, ACT_DIM], f32, tag="ot")
                        nc.vector.tensor_tensor(ot[:], pss[t][:], bdec_bc[:],
                                                op=mybir.AluOpType.add)
                        nc.sync.dma_start(
                            xhat_d.ap()[t * 128:(t + 1) * 128, :], ot[:])

    nc.compile()
    return nc


def _get_program(k: int, with_benc: bool):
    key = (k, with_benc)
    if key not in _cache:
        _cache[key] = _build(k, with_benc)
    return _cache[key]


def _host_repair(out, rows, x, W_enc, b_enc, W_dec, b_dec, k):
    for r in rows:
        pre = (x[r] - b_dec) @ W_enc.T + b_enc
        acts = np.maximum(pre, 0.0)
        idx = np.argsort(-acts, kind="stable")[:k]
        enc = np.zeros_like(acts)
        enc[idx] = acts[idx]
        out[r] = enc @ W_dec.T + b_dec


def run(inputs, trace=False):
    from concourse.bass_utils import run_bass_kernel_spmd

    x = np.asarray(inputs["x"], dtype=np.float32)
    W_enc = np.asarray(inputs["W_enc"], dtype=np.float32)
    b_enc = np.asarray(inputs["b_enc"], dtype=np.float32)
    W_dec = np.asarray(inputs["W_dec"], dtype=np.float32)
    b_dec = np.asarray(inputs["b_dec"], dtype=np.float32)
    k = int(np.asarray(inputs["k"]))
    assert x.shape == (BATCH, ACT_DIM) and W_enc.shape == (DICT, ACT_DIM)
    assert 1 <= k <= CANDW - 8

    with_benc = bool(np.any(b_enc))
    nc = _get_program(k, with_benc)

    xT = np.ascontiguousarray((x - b_dec).T, dtype=np.float32)
    wencT = np.ascontiguousarray(W_enc.T, dtype=np.float32)
    wdecT = np.ascontiguousarray(W_dec.T).astype(BF16)
    bdec_row = np.ascontiguousarray(b_dec.reshape(1, ACT_DIM))

    in_maps = []
    for c in range(NCORES):
        m = {
            "xT": np.ascontiguousarray(xT[:, c * ROWS:(c + 1) * ROWS]),
            "wencT": wencT,
            "wdecT": wdecT,
            "bdec": bdec_row,
        }
        if with_benc:
            m["benc"] = np.ascontiguousarray(b_enc.reshape(1, DICT))
        in_maps.append(m)

    res = run_bass_kernel_spmd(nc, in_maps, core_ids=list(range(NCORES)),
                               trace=trace)

    out = np.empty((BATCH, ACT_DIM), dtype=np.float32)
    flagged = []
    for c in range(NCORES):
        out[c * ROWS:(c + 1) * ROWS] = res.results[c]["xhat"]
        fl = res.results[c]["flags"]          # [128, NT]
        for t in range(NT):
            for p in np.nonzero(fl[:, t] > 0)[0]:
                flagged.append(c * ROWS + t * 128 + int(p))
    if flagged:
        _host_repair(out, flagged, x, W_enc, b_enc, W_dec, b_dec, k)
    return out, res, flagged


def kernel(**inputs) -> np.ndarray:
    out, _, _ = run(inputs)
    return out


# revision 3
# speedup vs baseline: 1.3339x; 1.3339x over previous
"""TopK sparse autoencoder forward pass on 8 TRN2 NeuronCores.

Data-parallel over the batch: each core owns 512 rows and runs an identical
program (SPMD).  Per core:

  A. encode:  acts = relu((x - b_dec) @ W_enc.T + b_enc)
     - computed as a 3-term bf16 hi/lo split (xh@Wh + xh@Wl + xl@Wh) which
       carries ~fp32 precision at 3/4 the PE cost of native fp32 matmul
       (fp32 lowers to 2 half-rate matmuls = 4x bf16 cost on TRN2)
     - W_enc hi/lo streamed from HBM once; fp32 acts spilled to DRAM
     - per-256-chunk top-8 candidates extracted from drain bounces (DVE max8)
  B. topk:    exact top-k threshold tau from the candidate array via
     iterated max8 + match_replace; exactness flag per row
  C. mask:    enc = (acts >= tau) * acts, cast bf16, DMA-transposed to [F, B]
  D. decode:  x_hat = enc @ W_dec.T + b_dec   [bf16, encoded-stationary]
     - C and D run block-pipelined over 2048-wide F blocks for all 4 row
       tiles at once, so W_dec streams exactly once per core

The per-chunk top-8 candidate set provably contains the true top-k unless
some 256-wide chunk holds >8 of the top-k values; that condition is detected
on-device (flag = chunk-8th-largest > tau) and the handful of flagged rows
(expected: zero) are recomputed exactly on the host.
"""

import numpy as np
import ml_dtypes

ACT_DIM = 768
DICT = 24576
BATCH = 4096
NCORES = 8
ROWS = BATCH // NCORES          # 512 rows per core
NT = ROWS // 128                # 4 row-tiles per core
CH = 256                        # stage-1 chunk width
NCH = DICT // CH                # 96 chunks
CANDW = NCH * 8                 # 768 candidates per row
NEG = -1.0e30
BF16 = ml_dtypes.bfloat16
NA = ACT_DIM // 128             # 6 K-chunks

_cache = {}


def _build(k: int, with_benc: bool):
    import concourse.bass as bass
    import concourse.mybir as mybir
    from concourse import bacc
    from concourse import tile

    f32 = mybir.dt.float32
    bf16 = mybir.dt.bfloat16
    ROUNDS = (k + 7) // 8

    nc = bacc.Bacc("TRN2", target_bir_lowering=False, debug=False,
                   num_devices=NCORES)

    xh_d = nc.dram_tensor("xh", [ACT_DIM, ROWS], bf16, kind="ExternalInput")
    xl_d = nc.dram_tensor("xl", [ACT_DIM, ROWS], bf16, kind="ExternalInput")
    wh_d = nc.dram_tensor("wencH", [ACT_DIM, DICT], bf16, kind="ExternalInput")
    wl_d = nc.dram_tensor("wencL", [ACT_DIM, DICT], bf16, kind="ExternalInput")
    wdecT_d = nc.dram_tensor("wdecT", [DICT, ACT_DIM], bf16, kind="ExternalInput")
    bdec_d = nc.dram_tensor("bdec", [1, ACT_DIM], f32, kind="ExternalInput")
    if with_benc:
        benc_d = nc.dram_tensor("benc", [1, DICT], f32, kind="ExternalInput")
    xhat_d = nc.dram_tensor("xhat", [ROWS, ACT_DIM], f32, kind="ExternalOutput")
    flags_d = nc.dram_tensor("flags", [128, NT], f32, kind="ExternalOutput")
    acts_spill = nc.dram_tensor("acts_spill", [NT, 128, DICT], f32)

    NSC = DICT // 512           # 48 encode column-chunks
    NBLK = DICT // 2048         # 12 C/D blocks
    NF = DICT // 128            # 192 decoder f-chunks

    with tile.TileContext(nc) as tc:
        with tc.tile_pool(name="const", bufs=1) as constp, \
             tc.tile_pool(name="cand", bufs=NT) as candp, \
             tc.tile_pool(name="small", bufs=4 * NT + 4) as smallp:

            bdec_row = constp.tile([1, ACT_DIM], f32)
            nc.sync.dma_start(bdec_row[:], bdec_d.ap())
            bdec_bc = constp.tile([128, ACT_DIM], f32)
            nc.gpsimd.partition_broadcast(bdec_bc[:], bdec_row[:])
            if with_benc:
                benc_row = constp.tile([1, DICT], f32)
                nc.sync.dma_start(benc_row[:], benc_d.ap())

            flags_sb = constp.tile([128, NT], f32)
            cands = [candp.tile([128, CANDW], f32, tag="cand", name=f"cand{t}")
                     for t in range(NT)]
            taus = [smallp.tile([128, 1], f32, tag="tau", name=f"tau{t}")
                    for t in range(NT)]

            # ---------------- Phase A: encode + spill + stage-1 ----------
            with tc.tile_pool(name="xt", bufs=1) as xtp, \
                 tc.tile_pool(name="wenc", bufs=4) as wencp, \
                 tc.tile_pool(name="bounce", bufs=6) as bouncep, \
                 tc.tile_pool(name="encpsum", bufs=6, space="PSUM") as encpsp, \
                 tc.tile_pool(name="bencbc", bufs=2) as bencbcp:

                xh_sb = xtp.tile([128, NA, ROWS], bf16)
                xl_sb = xtp.tile([128, NA, ROWS], bf16)
                nc.sync.dma_start(
                    xh_sb[:], xh_d.ap().rearrange("(a p) r -> p a r", p=128))
                nc.sync.dma_start(
                    xl_sb[:], xl_d.ap().rearrange("(a p) r -> p a r", p=128))

                for sc in range(NSC):
                    whch = wencp.tile([128, NA, 512], bf16, tag="wh",
                                      name=f"wh{sc}")
                    wlch = wencp.tile([128, NA, 512], bf16, tag="wl",
                                      name=f"wl{sc}")
                    nc.sync.dma_start(
                        whch[:],
                        wh_d.ap()[:, sc * 512:(sc + 1) * 512]
                        .rearrange("(a p) c -> p a c", p=128))
                    nc.sync.dma_start(
                        wlch[:],
                        wl_d.ap()[:, sc * 512:(sc + 1) * 512]
                        .rearrange("(a p) c -> p a c", p=128))
                    if with_benc:
                        bb = bencbcp.tile([128, 512], f32, tag="bb")
                        nc.gpsimd.partition_broadcast(
                            bb[:], benc_row[0:1, sc * 512:(sc + 1) * 512])
                    for t in range(NT):
                        ps = encpsp.tile([128, 512], f32, tag="eps")
                        rt = slice(t * 128, (t + 1) * 128)
                        n_mm = 3 * NA
                        i = 0
                        for a in range(NA):
                            # xh @ Wh_a ; xh @ Wl_a  (shared ldweights source)
                            for w in (whch, wlch):
                                nc.tensor.matmul(
                                    ps[:], xh_sb[:, a, rt], w[:, a, :],
                                    start=(i == 0), stop=(i == n_mm - 1))
                                i += 1
                        for a in range(NA):
                            nc.tensor.matmul(
                                ps[:], xl_sb[:, a, rt], whch[:, a, :],
                                start=(i == 0), stop=(i == n_mm - 1))
                            i += 1
                        bo = bouncep.tile([128, 512], f32, tag="bo")
                        if with_benc:
                            nc.vector.tensor_tensor(bo[:], ps[:], bb[:],
                                                    op=mybir.AluOpType.add)
                            nc.scalar.activation(
                                bo[:], bo[:], mybir.ActivationFunctionType.Relu)
                        else:
                            nc.scalar.activation(
                                bo[:], ps[:], mybir.ActivationFunctionType.Relu)
                        nc.sync.dma_start(
                            acts_spill.ap()[t, :, sc * 512:(sc + 1) * 512], bo[:])
                        for cc in range(512 // CH):
                            c = sc * (512 // CH) + cc
                            nc.vector.max(
                                cands[t][:, c * 8:(c + 1) * 8],
                                bo[:, cc * CH:(cc + 1) * CH])

            # ---------------- Phase B: exact top-k threshold -------------
            for t in range(NT):
                c8 = smallp.tile([128, 1], f32, tag="c8", name=f"c8_{t}")
                cand3 = cands[t][:].rearrange("p (c e) -> p c e", e=8)
                nc.vector.tensor_reduce(c8[:], cand3[:, :, 7:8],
                                        axis=mybir.AxisListType.XY,
                                        op=mybir.AluOpType.max)
                topv = smallp.tile([128, 8 * ROUNDS], f32, tag="topv",
                                   name=f"topv{t}")
                for r in range(ROUNDS):
                    nc.vector.max(topv[:, r * 8:(r + 1) * 8], cands[t][:])
                    if r < ROUNDS - 1:
                        nc.vector.match_replace(
                            cands[t][:], topv[:, r * 8:(r + 1) * 8],
                            cands[t][:], NEG)
                nc.vector.tensor_copy(taus[t][:], topv[:, k - 1:k])
                nc.vector.tensor_tensor(flags_sb[:, t:t + 1], c8[:], taus[t][:],
                                        op=mybir.AluOpType.is_gt)

            # -------- Phases C+D: block-pipelined mask/transpose/decode --
            with tc.tile_pool(name="actsc", bufs=6) as actscp, \
                 tc.tile_pool(name="encb", bufs=4) as encbp, \
                 tc.tile_pool(name="enct", bufs=3 * NT) as enctp, \
                 tc.tile_pool(name="wdec", bufs=3) as wdecp, \
                 tc.tile_pool(name="decpsum", bufs=NT, space="PSUM") as decpsp, \
                 tc.tile_pool(name="outsb", bufs=2) as outp:

                pss = [decpsp.tile([128, ACT_DIM], f32, tag="dps",
                                   name=f"dps{t}") for t in range(NT)]
                for blk in range(NBLK):
                    ets = []
                    for t in range(NT):
                        ac = actscp.tile([128, 2048], f32, tag="ac",
                                         name=f"ac{t}_{blk}")
                        nc.sync.dma_start(
                            ac[:],
                            acts_spill.ap()[t, :, blk * 2048:(blk + 1) * 2048])
                        eb = encbp.tile([128, 2048], bf16, tag="eb",
                                        name=f"eb{t}_{blk}")
                        nc.vector.scalar_tensor_tensor(
                            eb[:], ac[:], taus[t][:, 0:1], ac[:],
                            op0=mybir.AluOpType.is_ge,
                            op1=mybir.AluOpType.mult)
                        et = enctp.tile([128, 16, 128], bf16, tag="enct",
                                        name=f"et{t}_{blk}")
                        nc.sync.dma_start_transpose(et[:], eb[:])
                        ets.append(et)
                    for g in range(2):
                        wd = wdecp.tile([128, 8, ACT_DIM], bf16, tag="wd",
                                        name=f"wd{blk}_{g}")
                        f0 = blk * 2048 + g * 1024
                        nc.sync.dma_start(
                            wd[:],
                            wdecT_d.ap()[f0:f0 + 1024, :]
                            .rearrange("(c p) a -> p c a", p=128))
                        for t in range(NT):
                            for j in range(8):
                                f = blk * 16 + g * 8 + j
                                lhsT = ets[t][:, g * 8 + j, :]
                                st = (f == 0)
                                sp = (f == NF - 1)
                                nc.tensor.matmul(
                                    pss[t][:, 0:512], lhsT, wd[:, j, 0:512],
                                    start=st, stop=sp)
                                nc.tensor.matmul(
                                    pss[t][:, 512:ACT_DIM], lhsT,
                                    wd[:, j, 512:ACT_DIM],
                                    start=st, stop=sp)
                for t in range(NT):
                    ot = outp.tile([128, ACT_DIM], f32, tag="ot",
                                   name=f"ot{t}")
                    nc.vector.tensor_tensor(ot[:], pss[t][:], bdec_bc[:],
                                            op=mybir.AluOpType.add)
                    nc.sync.dma_start(
                        xhat_d.ap()[t * 128:(t + 1) * 128, :], ot[:])
                nc.sync.dma_start(flags_d.ap(), flags_sb[:])

    nc.compile()
    return nc


def _get_program(k: int, with_benc: bool):
    key = (k, with_benc)
    if key not in _cache:
        _cache[key] = _build(k, with_benc)
    return _cache[key]


def _host_repair(out, rows, x, W_enc, b_enc, W_dec, b_dec, k):
    for r in rows:
        pre = (x[r] - b_dec) @ W_enc.T + b_enc
        acts = np.maximum(pre, 0.0)
        idx = np.argsort(-acts, kind="stable")[:k]
        enc = np.zeros_like(acts)
        enc[idx] = acts[idx]
        out[r] = enc @ W_dec.T + b_dec


def run(inputs, trace=False):
    from concourse.bass_utils import run_bass_kernel_spmd

    x = np.asarray(inputs["x"], dtype=np.float32)
    W_enc = np.asarray(inputs["W_enc"], dtype=np.float32)
    b_enc = np.asarray(inputs["b_enc"], dtype=np.float32)
    W_dec = np.asarray(inputs["W_dec"], dtype=np.float32)
    b_dec = np.asarray(inputs["b_dec"], dtype=np.float32)
    k = int(np.asarray(inputs["k"]))
    assert x.shape == (BATCH, ACT_DIM) and W_enc.shape == (DICT, ACT_DIM)
    assert 1 <= k <= CANDW - 8

    with_benc = bool(np.any(b_enc))
    nc = _get_program(k, with_benc)

    xT = np.ascontiguousarray((x - b_dec).T, dtype=np.float32)
    xTh = xT.astype(BF16)
    xTl = (xT - xTh.astype(np.float32)).astype(BF16)
    wencT = np.ascontiguousarray(W_enc.T, dtype=np.float32)
    wencH = wencT.astype(BF16)
    wencL = (wencT - wencH.astype(np.float32)).astype(BF16)
    wdecT = np.ascontiguousarray(W_dec.T).astype(BF16)
    bdec_row = np.ascontiguousarray(b_dec.reshape(1, ACT_DIM))

    in_maps = []
    for c in range(NCORES):
        sl = slice(c * ROWS, (c + 1) * ROWS)
        m = {
            "xh": np.ascontiguousarray(xTh[:, sl]),
            "xl": np.ascontiguousarray(xTl[:, sl]),
            "wencH": wencH,
            "wencL": wencL,
            "wdecT": wdecT,
            "bdec": bdec_row,
        }
        if with_benc:
            m["benc"] = np.ascontiguousarray(b_enc.reshape(1, DICT))
        in_maps.append(m)

    res = run_bass_kernel_spmd(nc, in_maps, core_ids=list(range(NCORES)),
                               trace=trace)

    out = np.empty((BATCH, ACT_DIM), dtype=np.float32)
    flagged = []
    for c in range(NCORES):
        out[c * ROWS:(c + 1) * ROWS] = res.results[c]["xhat"]
        fl = res.results[c]["flags"]          # [128, NT]
        for t in range(NT):
            for p in np.nonzero(fl[:, t] > 0)[0]:
                flagged.append(c * ROWS + t * 128 + int(p))
    if flagged:
        _host_repair(out, flagged, x, W_enc, b_enc, W_dec, b_dec, k)
    return out, res, flagged


def kernel(**inputs) -> np.ndarray:
    out, _, _ = run(inputs)
    return out
